# revision 13
# baseline (speedup 1.0000x reference)
"""Trainium2 Bass kernel for DepthwiseSeparableConv3d (inference).

Problem: x[2,48,48,48,64] -> dw3x3x3 depthwise + BN + ReLU -> 1x1x1 conv
(64->128) + BN + ReLU -> z[2,48,48,48,128], all f32.

Strategy (8 NeuronCores, data-parallel over (b, h-quarter) slabs):
 - Each core owns batch b = core//4 and h-rows [12*hq, 12*hq+12) for
   hq = core%4, full D and W, with SAME-pad halos baked in on host.
 - Depthwise conv runs on TensorE as a 2D block-Toeplitz matmul:
   K = 128 partitions = (2 channels x 8x8 (h,w) input patch),
   M = 72 partitions  = (2 channels x 6x6 (h,w) output patch).
   The 9 (dy,dx) taps live in the Toeplitz weight; the 3 dz taps are
   PSUM-accumulated matmuls against d-shifted views of the same SBUF
   tile.  3 passes instead of the 9 a 1D fold needs.
 - BN1 scale a1 is folded into the Toeplitz weights, so BN1+ReLU is
   relu(psum + c1): one ScalarE activation per (pair, phase).  Same
   fold for BN2 into the pointwise weights; BN2+ReLU is a single
   add+max tensor_scalar split across DVE and ScalarE.
 - A per-(8-pair group) SBUF->SBUF DMA on the GpSimd SWDGE queue
   regroups (c2,ho,wo)-partitions into pure-channel partitions.
 - Two d-phases (out d 0..31 | 32..47) pipeline PW of phase A under
   DW of phase B; PW chunks are interleaved between DW pairs.
 - Output stays [f, positions] on device; host transposes to NDHWC.
"""

import sys

for _p in ("/opt/trn_rl_repo", "/opt/pypackages"):
    if _p not in sys.path:
        sys.path.insert(0, _p)

import numpy as np
import ml_dtypes

import concourse.bass as bass
import concourse.tile as tile
from concourse import bacc, mybir
from concourse.bass_utils import run_bass_kernel_spmd

# ----- problem constants (hardcoded per spec) -----
B, D, H, W, C, F = 2, 48, 48, 48, 64, 128
EPS = 1e-3
N_CORES = 8
HQ = H // 4                       # 12 h-rows per core
NP = C // 2                       # 32 channel-pairs
PO, PI = 6, 8                     # patch out/in edge (ho,wo)/(hy,wx)
TH = (HQ + 2 - PI) // PO + 1      # 2 h-tiles per core slab
TW = (W + 2 - PI) // PO + 1      # 8 w-tiles
MP = 2 * PO * PO                  # 72 output partitions
KP = 2 * PI * PI                  # 128 input partitions
DI = D + 2                        # 50 padded d slices
NTT = TH * TW                     # 16 (th,tw) tiles
DA, DB = 32, 16                   # d-phase split (outputs)
NA, NB = DA * NTT, DB * NTT       # 512 / 256 moving cols per (pair,dz)
GRP = 8                           # pairs per regroup group
NG = NP // GRP                    # 4 groups
ZC = 512                          # pointwise chunk (PSUM cols, 1 bank)
NCA, NCB = 36 * NA // ZC, 36 * NB // ZC   # 36 / 18 pw chunks per phase
NPOS = 36 * (NA + NB)             # 27648 positions per core

BF16 = mybir.dt.bfloat16
F32 = mybir.dt.float32
RELU = mybir.ActivationFunctionType.Relu
ADD = mybir.AluOpType.add
MAX = mybir.AluOpType.max

_COMPILED = None


def _build_bass():
    nc = bacc.Bacc("TRN2", target_bir_lowering=False, debug=False,
                   num_devices=N_CORES)

    xt_d = nc.dram_tensor("xt", [NP, KP, DI, NTT], BF16,
                          kind="ExternalInput").ap()
    wt_d = nc.dram_tensor("wt", [KP, NP, 3, MP], BF16,
                          kind="ExternalInput").ap()
    pw_d = nc.dram_tensor("pwk", [C, F], BF16, kind="ExternalInput").ap()
    c1_d = nc.dram_tensor("c1b", [MP, NP], F32, kind="ExternalInput").ap()
    c2_d = nc.dram_tensor("c2b", [F, 1], F32, kind="ExternalInput").ap()
    z_d = nc.dram_tensor("z", [F, NPOS], BF16, kind="ExternalOutput").ap()

    PH = ((0, DA, NA, NCA), (DA, DB, NB, NCB))

    with tile.TileContext(nc) as tc:
        with (
            tc.tile_pool(name="consts", bufs=1) as consts,
            tc.tile_pool(name="xt", bufs=NG) as xt_pool,
            tc.tile_pool(name="Y", bufs=1) as Y_pool,
            tc.tile_pool(name="yg", bufs=4) as yg_pool,
            tc.tile_pool(name="zbuf", bufs=4) as z_pool,
        ):
            pw_sb = consts.tile([C, F], BF16)
            c1_sb = consts.tile([MP, NP], F32)
            c2_sb = consts.tile([F, 1], F32)
            wt_sb = consts.tile([KP, NP, 3, MP], BF16)

            xg = [xt_pool.tile([KP, GRP, DI, NTT], BF16, tag="xg",
                               name=f"xg_{g}")
                  for g in range(NG)]

            # input DMAs: x on the SP ring (phase-A d-range first, in
            # group order so compute can start after the first group),
            # weights/consts on the ACT ring
            xsrc = xt_d.rearrange("(g p) k d t -> k g p d t", g=NG)
            nc.scalar.dma_start(wt_sb[:, 0:GRP], wt_d[:, 0:GRP])
            nc.sync.dma_start(xg[0][:, :, 0:DA + 2], xsrc[:, 0, :, 0:DA + 2])
            nc.scalar.dma_start(c1_sb[:], c1_d[:])
            nc.scalar.dma_start(c2_sb[:], c2_d[:])
            nc.scalar.dma_start(pw_sb[:], pw_d[:])
            nc.sync.dma_start(xg[1][:, :, 0:DA + 2], xsrc[:, 1, :, 0:DA + 2])
            nc.scalar.dma_start(wt_sb[:, GRP:NP], wt_d[:, GRP:NP])
            for g in (2, 3):
                nc.sync.dma_start(xg[g][:, :, 0:DA + 2],
                                  xsrc[:, g, :, 0:DA + 2])
            for g in range(NG):
                nc.sync.dma_start(xg[g][:, :, DA + 2:DI],
                                  xsrc[:, g, :, DA + 2:DI])

            # depthwise output, channel-partition layout, per phase
            Yt = [Y_pool.tile([C, 36, n], BF16, tag=f"Y{i}", name=f"Y{i}")
                  for i, (_, _, n, _) in enumerate(PH)]

            with (
                tc.tile_pool(name="psdw", bufs=4, space="PSUM") as ps_pool,
                tc.tile_pool(name="pspw", bufs=4, space="PSUM") as pw_pool,
            ):
                pw_fifo = []
                zq = {"n": 0}

                def flush_z():
                    k = zq["n"]
                    if not k:
                        return
                    nc.sync.dma_start(
                        z_d[:, zq["off"]:zq["off"] + k * ZC],
                        zq["t"][:, 0:k].rearrange("f s r -> f (s r)"))
                    zq["n"] = 0

                def emit_pw(ph, q, tail=False):
                    off = (0 if ph == 0 else 36 * NA) + q * ZC
                    Yv = Yt[ph][:].rearrange("c a b -> c (a b)")
                    pps = pw_pool.tile([F, ZC], F32, tag="pwps",
                                       name=f"pps_{ph}_{q}")
                    nc.tensor.matmul(pps[:], pw_sb[:],
                                     Yv[:, q * ZC:(q + 1) * ZC],
                                     start=True, stop=True)
                    if zq["n"] == 0:
                        zq["t"] = z_pool.tile([F, 4, ZC], BF16, tag="zt",
                                              name=f"zt_{ph}_{q}")
                        zq["off"] = off
                    s = zq["n"]
                    zt = zq["t"]
                    on_act = (q % 2 == 0) if tail else (s == 3)
                    if on_act:
                        nc.scalar.activation(zt[:, s], pps[:], RELU,
                                             bias=c2_sb[:, 0:1])
                    else:
                        nc.vector.tensor_scalar(zt[:, s], pps[:],
                                                c2_sb[:, 0:1], 0.0,
                                                ADD, MAX)
                    zq["n"] = s + 1
                    if zq["n"] == 4:
                        flush_z()

                for ph, (d0, dn, n, nch) in enumerate(PH):
                    for g in range(NG):
                        for pg in range(GRP):
                            p = g * GRP + pg
                            ps = ps_pool.tile([MP, NA], F32, tag="ps",
                                              name=f"ps_{ph}_{p}")
                            for dz in range(3):
                                rhs = xg[g][:, pg, d0 + dz:d0 + dz + dn]
                                nc.tensor.matmul(
                                    ps[:, 0:n], wt_sb[:, p, dz], rhs,
                                    start=(dz == 0), stop=(dz == 2))
                            ygp = yg_pool.tile([MP, n], BF16,
                                               tag=f"yg{ph}",
                                               name=f"yg_{ph}_{p}")
                            nc.scalar.activation(
                                ygp[:], ps[:, 0:n], RELU,
                                bias=c1_sb[:, p:p + 1])
                            # regroup (c2,ho,wo)->channel partitions;
                            # split across the idle Pool SWDGE queue
                            # and the SP queue
                            eng = nc.gpsimd if p % 2 == 0 else nc.sync
                            eng.dma_start(Yt[ph][2 * p: 2 * p + 2], ygp[:])
                            # interleave prev-phase PW under this DW
                            if pg % 2 == 1 and pw_fifo:
                                for _ in range(3):
                                    if pw_fifo:
                                        emit_pw(*pw_fifo.pop(0))
                    for q in range(nch):
                        pw_fifo.append((ph, q))
                    if ph == 1:
                        while pw_fifo:
                            emit_pw(*pw_fifo.pop(0), tail=True)
                        flush_z()

    nc.compile()
    return nc


def _prep_inputs(x, dw_kernel, dw_bias, bn1_gamma, bn1_beta, bn1_mean,
                 bn1_var, pw_kernel, pw_bias, bn2_gamma, bn2_beta, bn2_mean,
                 bn2_var):
    """Build per-core input maps (numpy only, off the device clock)."""
    x = np.asarray(x, np.float32)
    dw = np.asarray(dw_kernel, np.float32)[:, :, :, 0, :]     # [3,3,3,C]
    a1 = np.asarray(bn1_gamma, np.float32) / np.sqrt(
        np.asarray(bn1_var, np.float32) + EPS)
    c1 = a1 * (np.asarray(dw_bias, np.float32)
               - np.asarray(bn1_mean, np.float32)) \
        + np.asarray(bn1_beta, np.float32)
    a2 = np.asarray(bn2_gamma, np.float32) / np.sqrt(
        np.asarray(bn2_var, np.float32) + EPS)
    c2 = a2 * (np.asarray(pw_bias, np.float32)
               - np.asarray(bn2_mean, np.float32)) \
        + np.asarray(bn2_beta, np.float32)

    # Toeplitz weights [KP, NP, 3, MP], a1 prefolded, k-major
    aw = dw * a1[None, None, None, :]                         # [3,3,3,C]
    wt = np.zeros((2, PI, PI, NP, 3, 2, PO, PO), np.float32)
    c2i = np.arange(2)[:, None, None]
    hoi = np.arange(PO)[None, :, None]
    woi = np.arange(PO)[None, None, :]
    for dy in range(3):
        for dx in range(3):
            # value for [c2, ho, wo, p, dz] = aw[dz, dy, dx, 2p+c2]
            val = aw[:, dy, dx, :].reshape(3, NP, 2)          # [dz, p, c2]
            val = val.transpose(2, 1, 0)[:, None, None]       # [2,1,1,NP,3]
            wt[c2i, dy + hoi, dx + woi, :, :, c2i, hoi, woi] = val
    wt = wt.reshape(KP, NP, 3, MP).astype(ml_dtypes.bfloat16)

    # c1b[(c2,ho,wo), p] = c1[2p+c2]
    c1b = np.ascontiguousarray(
        np.repeat(c1.reshape(NP, 2).T, PO * PO, axis=0).reshape(MP, NP)
    ).astype(np.float32)
    pwk = (np.asarray(pw_kernel, np.float32)
           * a2[None, :]).astype(ml_dtypes.bfloat16)
    c2b = c2.reshape(F, 1).astype(np.float32)

    # x padded once globally: [B, D+2, H+2, W+2, C]
    xp = np.zeros((B, D + 2, H + 2, W + 2, C), np.float32)
    xp[:, 1:-1, 1:-1, 1:-1, :] = x

    widx = (np.arange(TW)[:, None] * PO + np.arange(PI)[None, :])  # [8,8]
    hidx = (np.arange(TH)[:, None] * PO + np.arange(PI)[None, :])  # [2,8]
    in_maps = []
    for core in range(N_CORES):
        b, hq = core // 4, core % 4
        slab = xp[b, :, hq * HQ: hq * HQ + HQ + 2]        # [50, 14, 50, C]
        t = slab[:, :, widx.ravel()].reshape(DI, HQ + 2, TW, PI, C)
        t = t[:, hidx.ravel()].reshape(DI, TH, PI, TW, PI, C)
        # [d, th, hy, tw, wx, c] -> [p, c2, hy, wx, d, th, tw]
        t = t.transpose(5, 2, 4, 0, 1, 3)                 # [C,hy,wx,d,th,tw]
        xt = t.reshape(NP, 2, PI, PI, DI, NTT).reshape(NP, KP, DI, NTT)
        in_maps.append({
            "xt": np.ascontiguousarray(xt).astype(ml_dtypes.bfloat16),
            "wt": wt, "pwk": pwk, "c1b": c1b, "c2b": c2b,
        })
    return in_maps


def _gather_output(results):
    z = np.empty((B, D, H, W, F), np.float32)
    for core in range(N_CORES):
        b, hq = core // 4, core % 4
        zc = np.asarray(results[core]["z"], dtype=np.float32)  # [F, NPOS]
        za = zc[:, :36 * NA].reshape(F, PO, PO, DA, TH, TW)
        zb = zc[:, 36 * NA:].reshape(F, PO, PO, DB, TH, TW)
        for z_ph, d0, dn in ((za, 0, DA), (zb, DA, DB)):
            # [f, ho, wo, d, th, tw] -> [d, th, ho, tw, wo, f]
            v = z_ph.transpose(3, 4, 1, 5, 2, 0)
            z[b, d0:d0 + dn, hq * HQ: hq * HQ + HQ] = \
                v.reshape(dn, HQ, W, F)
    return z


def kernel(**inputs):
    global _COMPILED
    if _COMPILED is None:
        _COMPILED = _build_bass()
    in_maps = _prep_inputs(**inputs)
    res = run_bass_kernel_spmd(_COMPILED, in_maps,
                               core_ids=list(range(N_CORES)))
    return _gather_output(res.results)


if __name__ == "__main__":
    pass


# revision 17
# speedup vs baseline: 1.0216x; 1.0216x over previous
"""Trainium2 Bass kernel for DepthwiseSeparableConv3d (inference).

Problem: x[2,48,48,48,64] -> dw3x3x3 depthwise + BN + ReLU -> 1x1x1 conv
(64->128) + BN + ReLU -> z[2,48,48,48,128], all f32.

Strategy (8 NeuronCores, data-parallel over (b, h-quarter) slabs):
 - Each core owns batch b = core//4 and h-rows [12*hq, 12*hq+12) for
   hq = core%4, full D and W, with SAME-pad halos baked in on host.
 - Depthwise conv runs on TensorE as a 2D block-Toeplitz matmul:
   K = 128 partitions = (2 channels x 8x8 (h,w) input patch),
   M = 72 partitions  = (2 channels x 6x6 (h,w) output patch).
   The 9 (dy,dx) taps live in the Toeplitz weight; the 3 dz taps are
   PSUM-accumulated matmuls against d-shifted views of the same SBUF
   tile.  3 passes instead of the 9 a 1D fold needs.
 - BN1 scale a1 is folded into the Toeplitz weights, so BN1+ReLU is
   relu(psum + c1): one ScalarE activation per (pair, phase).  Same
   fold for BN2 into the pointwise weights; BN2+ReLU is a single
   add+max tensor_scalar split across DVE and ScalarE.
 - A per-(8-pair group) SBUF->SBUF DMA on the GpSimd SWDGE queue
   regroups (c2,ho,wo)-partitions into pure-channel partitions.
 - Two d-phases (out d 0..31 | 32..47) pipeline PW of phase A under
   DW of phase B; PW chunks are interleaved between DW pairs.
 - Output stays [f, positions] on device; host transposes to NDHWC.
"""

import sys

for _p in ("/opt/trn_rl_repo", "/opt/pypackages"):
    if _p not in sys.path:
        sys.path.insert(0, _p)

import numpy as np
import ml_dtypes

import concourse.bass as bass
import concourse.tile as tile
from concourse import bacc, mybir
from concourse.bass_utils import run_bass_kernel_spmd

# ----- problem constants (hardcoded per spec) -----
B, D, H, W, C, F = 2, 48, 48, 48, 64, 128
EPS = 1e-3
N_CORES = 8
HQ = H // 4                       # 12 h-rows per core
NP = C // 2                       # 32 channel-pairs
PO, PI = 6, 8                     # patch out/in edge (ho,wo)/(hy,wx)
TH = (HQ + 2 - PI) // PO + 1      # 2 h-tiles per core slab
TW = (W + 2 - PI) // PO + 1      # 8 w-tiles
MP = 2 * PO * PO                  # 72 output partitions
KP = 2 * PI * PI                  # 128 input partitions
DI = D + 2                        # 50 padded d slices
NTT = TH * TW                     # 16 (th,tw) tiles
DA, DB = 32, 16                   # d-phase split (outputs)
NA, NB = DA * NTT, DB * NTT       # 512 / 256 moving cols per (pair,dz)
GRP = 8                           # pairs per regroup group
NG = NP // GRP                    # 4 groups
ZC = 512                          # pointwise chunk (PSUM cols, 1 bank)
NCA, NCB = 36 * NA // ZC, 36 * NB // ZC   # 36 / 18 pw chunks per phase
NPOS = 36 * (NA + NB)             # 27648 positions per core

BF16 = mybir.dt.bfloat16
F32 = mybir.dt.float32
RELU = mybir.ActivationFunctionType.Relu
ADD = mybir.AluOpType.add
MAX = mybir.AluOpType.max

_COMPILED = None


def _build_bass():
    nc = bacc.Bacc("TRN2", target_bir_lowering=False, debug=False,
                   num_devices=N_CORES)

    xt_d = nc.dram_tensor("xt", [NP, KP, DI, NTT], BF16,
                          kind="ExternalInput").ap()
    wt_d = nc.dram_tensor("wt", [KP, NP, 3, MP], BF16,
                          kind="ExternalInput").ap()
    pw_d = nc.dram_tensor("pwk", [C, F], BF16, kind="ExternalInput").ap()
    c1_d = nc.dram_tensor("c1b", [MP, NP], F32, kind="ExternalInput").ap()
    c2_d = nc.dram_tensor("c2b", [F, 1], F32, kind="ExternalInput").ap()
    z_d = nc.dram_tensor("z", [F, NPOS], BF16, kind="ExternalOutput").ap()

    PH = ((0, DA, NA, NCA), (DA, DB, NB, NCB))

    with tile.TileContext(nc) as tc:
        with (
            tc.tile_pool(name="consts", bufs=1) as consts,
            tc.tile_pool(name="xt", bufs=NG) as xt_pool,
            tc.tile_pool(name="Y", bufs=1) as Y_pool,
            tc.tile_pool(name="yg", bufs=4) as yg_pool,
            tc.tile_pool(name="zbuf", bufs=4) as z_pool,
        ):
            pw_sb = consts.tile([C, F], BF16)
            c1_sb = consts.tile([MP, NP], F32)
            c2_sb = consts.tile([F, 1], F32)
            wt_sb = consts.tile([KP, NP, 3, MP], BF16)

            xg = [xt_pool.tile([KP, GRP, DI, NTT], BF16, tag="xg",
                               name=f"xg_{g}")
                  for g in range(NG)]

            # input DMAs: x on the SP ring (phase-A d-range first, in
            # group order so compute can start after the first group),
            # weights/consts on the ACT ring
            xsrc = xt_d.rearrange("(g p) k d t -> k g p d t", g=NG)
            nc.scalar.dma_start(wt_sb[:, 0:2], wt_d[:, 0:2])
            nc.sync.dma_start(xg[0][:, 0:2, 0:DA + 2],
                              xsrc[:, 0, 0:2, 0:DA + 2])
            nc.scalar.dma_start(c1_sb[:], c1_d[:])
            nc.scalar.dma_start(c2_sb[:], c2_d[:])
            nc.scalar.dma_start(pw_sb[:], pw_d[:])
            nc.sync.dma_start(xg[0][:, 2:GRP, 0:DA + 2],
                              xsrc[:, 0, 2:GRP, 0:DA + 2])
            nc.scalar.dma_start(wt_sb[:, 2:GRP], wt_d[:, 2:GRP])
            nc.sync.dma_start(xg[1][:, :, 0:DA + 2], xsrc[:, 1, :, 0:DA + 2])
            nc.scalar.dma_start(wt_sb[:, GRP:NP], wt_d[:, GRP:NP])
            for g in (2, 3):
                nc.sync.dma_start(xg[g][:, :, 0:DA + 2],
                                  xsrc[:, g, :, 0:DA + 2])
            for g in range(NG):
                nc.sync.dma_start(xg[g][:, :, DA + 2:DI],
                                  xsrc[:, g, :, DA + 2:DI])

            # depthwise output, channel-partition layout, per phase
            Yt = [Y_pool.tile([C, 36, n], BF16, tag=f"Y{i}", name=f"Y{i}")
                  for i, (_, _, n, _) in enumerate(PH)]

            with (
                tc.tile_pool(name="psdw", bufs=4, space="PSUM") as ps_pool,
                tc.tile_pool(name="pspw", bufs=4, space="PSUM") as pw_pool,
            ):
                pw_fifo = []
                zq = {"n": 0}

                def flush_z():
                    k = zq["n"]
                    if not k:
                        return
                    nc.sync.dma_start(
                        z_d[:, zq["off"]:zq["off"] + k * ZC],
                        zq["t"][:, 0:k].rearrange("f s r -> f (s r)"))
                    zq["n"] = 0

                def emit_pw(ph, q, tail=False):
                    off = (0 if ph == 0 else 36 * NA) + q * ZC
                    Yv = Yt[ph][:].rearrange("c a b -> c (a b)")
                    pps = pw_pool.tile([F, ZC], F32, tag="pwps",
                                       name=f"pps_{ph}_{q}")
                    nc.tensor.matmul(pps[:], pw_sb[:],
                                     Yv[:, q * ZC:(q + 1) * ZC],
                                     start=True, stop=True)
                    if zq["n"] == 0:
                        zq["t"] = z_pool.tile([F, 4, ZC], BF16, tag="zt",
                                              name=f"zt_{ph}_{q}")
                        zq["off"] = off
                    s = zq["n"]
                    zt = zq["t"]
                    on_act = (q % 2 == 0) if tail else (q % 3 == 2)
                    if on_act:
                        nc.scalar.activation(zt[:, s], pps[:], RELU,
                                             bias=c2_sb[:, 0:1])
                    else:
                        nc.vector.tensor_scalar(zt[:, s], pps[:],
                                                c2_sb[:, 0:1], 0.0,
                                                ADD, MAX)
                    zq["n"] = s + 1
                    if zq["n"] == 4:
                        flush_z()

                for ph, (d0, dn, n, nch) in enumerate(PH):
                    for g in range(NG):
                        for pg in range(GRP):
                            p = g * GRP + pg
                            ps = ps_pool.tile([MP, NA], F32, tag="ps",
                                              name=f"ps_{ph}_{p}")
                            for dz in range(3):
                                rhs = xg[g][:, pg, d0 + dz:d0 + dz + dn]
                                nc.tensor.matmul(
                                    ps[:, 0:n], wt_sb[:, p, dz], rhs,
                                    start=(dz == 0), stop=(dz == 2))
                            ygp = yg_pool.tile([MP, n], BF16,
                                               tag=f"yg{ph}",
                                               name=f"yg_{ph}_{p}")
                            nc.scalar.activation(
                                ygp[:], ps[:, 0:n], RELU,
                                bias=c1_sb[:, p:p + 1])
                            # regroup (c2,ho,wo)->channel partitions;
                            # split across the idle Pool SWDGE queue
                            # and the ACT queue (sync carries bulk x/z)
                            eng = nc.gpsimd if p % 2 == 0 else nc.scalar
                            eng.dma_start(Yt[ph][2 * p: 2 * p + 2], ygp[:])
                            # interleave prev-phase PW under this DW
                            if pg % 2 == 1 and pw_fifo:
                                for _ in range(3):
                                    if pw_fifo:
                                        emit_pw(*pw_fifo.pop(0))
                    for q in range(nch):
                        pw_fifo.append((ph, q))
                    if ph == 1:
                        while pw_fifo:
                            emit_pw(*pw_fifo.pop(0), tail=True)
                        flush_z()

    nc.compile()
    return nc


def _prep_inputs(x, dw_kernel, dw_bias, bn1_gamma, bn1_beta, bn1_mean,
                 bn1_var, pw_kernel, pw_bias, bn2_gamma, bn2_beta, bn2_mean,
                 bn2_var):
    """Build per-core input maps (numpy only, off the device clock)."""
    x = np.asarray(x, np.float32)
    dw = np.asarray(dw_kernel, np.float32)[:, :, :, 0, :]     # [3,3,3,C]
    a1 = np.asarray(bn1_gamma, np.float32) / np.sqrt(
        np.asarray(bn1_var, np.float32) + EPS)
    c1 = a1 * (np.asarray(dw_bias, np.float32)
               - np.asarray(bn1_mean, np.float32)) \
        + np.asarray(bn1_beta, np.float32)
    a2 = np.asarray(bn2_gamma, np.float32) / np.sqrt(
        np.asarray(bn2_var, np.float32) + EPS)
    c2 = a2 * (np.asarray(pw_bias, np.float32)
               - np.asarray(bn2_mean, np.float32)) \
        + np.asarray(bn2_beta, np.float32)

    # Toeplitz weights [KP, NP, 3, MP], a1 prefolded, k-major
    aw = dw * a1[None, None, None, :]                         # [3,3,3,C]
    wt = np.zeros((2, PI, PI, NP, 3, 2, PO, PO), np.float32)
    c2i = np.arange(2)[:, None, None]
    hoi = np.arange(PO)[None, :, None]
    woi = np.arange(PO)[None, None, :]
    for dy in range(3):
        for dx in range(3):
            # value for [c2, ho, wo, p, dz] = aw[dz, dy, dx, 2p+c2]
            val = aw[:, dy, dx, :].reshape(3, NP, 2)          # [dz, p, c2]
            val = val.transpose(2, 1, 0)[:, None, None]       # [2,1,1,NP,3]
            wt[c2i, dy + hoi, dx + woi, :, :, c2i, hoi, woi] = val
    wt = wt.reshape(KP, NP, 3, MP).astype(ml_dtypes.bfloat16)

    # c1b[(c2,ho,wo), p] = c1[2p+c2]
    c1b = np.ascontiguousarray(
        np.repeat(c1.reshape(NP, 2).T, PO * PO, axis=0).reshape(MP, NP)
    ).astype(np.float32)
    pwk = (np.asarray(pw_kernel, np.float32)
           * a2[None, :]).astype(ml_dtypes.bfloat16)
    c2b = c2.reshape(F, 1).astype(np.float32)

    # x padded once globally: [B, D+2, H+2, W+2, C]
    xp = np.zeros((B, D + 2, H + 2, W + 2, C), np.float32)
    xp[:, 1:-1, 1:-1, 1:-1, :] = x

    widx = (np.arange(TW)[:, None] * PO + np.arange(PI)[None, :])  # [8,8]
    hidx = (np.arange(TH)[:, None] * PO + np.arange(PI)[None, :])  # [2,8]
    in_maps = []
    for core in range(N_CORES):
        b, hq = core // 4, core % 4
        slab = xp[b, :, hq * HQ: hq * HQ + HQ + 2]        # [50, 14, 50, C]
        t = slab[:, :, widx.ravel()].reshape(DI, HQ + 2, TW, PI, C)
        t = t[:, hidx.ravel()].reshape(DI, TH, PI, TW, PI, C)
        # [d, th, hy, tw, wx, c] -> [p, c2, hy, wx, d, th, tw]
        t = t.transpose(5, 2, 4, 0, 1, 3)                 # [C,hy,wx,d,th,tw]
        xt = t.reshape(NP, 2, PI, PI, DI, NTT).reshape(NP, KP, DI, NTT)
        in_maps.append({
            "xt": np.ascontiguousarray(xt).astype(ml_dtypes.bfloat16),
            "wt": wt, "pwk": pwk, "c1b": c1b, "c2b": c2b,
        })
    return in_maps


def _gather_output(results):
    z = np.empty((B, D, H, W, F), np.float32)
    for core in range(N_CORES):
        b, hq = core // 4, core % 4
        zc = np.asarray(results[core]["z"], dtype=np.float32)  # [F, NPOS]
        za = zc[:, :36 * NA].reshape(F, PO, PO, DA, TH, TW)
        zb = zc[:, 36 * NA:].reshape(F, PO, PO, DB, TH, TW)
        for z_ph, d0, dn in ((za, 0, DA), (zb, DA, DB)):
            # [f, ho, wo, d, th, tw] -> [d, th, ho, tw, wo, f]
            v = z_ph.transpose(3, 4, 1, 5, 2, 0)
            z[b, d0:d0 + dn, hq * HQ: hq * HQ + HQ] = \
                v.reshape(dn, HQ, W, F)
    return z


def kernel(**inputs):
    global _COMPILED
    if _COMPILED is None:
        _COMPILED = _build_bass()
    in_maps = _prep_inputs(**inputs)
    res = run_bass_kernel_spmd(_COMPILED, in_maps,
                               core_ids=list(range(N_CORES)))
    return _gather_output(res.results)


if __name__ == "__main__":
    pass


# revision 18
# speedup vs baseline: 1.1345x; 1.1106x over previous
"""Trainium2 Bass kernel for DepthwiseSeparableConv3d (inference).

Problem: x[2,48,48,48,64] -> dw3x3x3 depthwise + BN + ReLU -> 1x1x1 conv
(64->128) + BN + ReLU -> z[2,48,48,48,128], all f32.

Strategy (8 NeuronCores, data-parallel over (b, h-quarter) slabs):
 - Each core owns batch b = core//4 and h-rows [12*hq, 12*hq+12) for
   hq = core%4, full D and W, with SAME-pad halos baked in on host.
 - Depthwise conv runs on TensorE as a 2D block-Toeplitz matmul:
   K = 120 partitions = (2 channels x 6x10 (h,w) input patch),
   M = 64 partitions  = (2 channels x 4x8 (h,w) output patch).
   The 9 (dy,dx) taps live in the Toeplitz weight; the 3 dz taps are
   PSUM-accumulated matmuls against d-shifted views of the same SBUF
   tile.  TWO channel-pairs share each PSUM tile via PE column tiling
   (tile_position cols 0/64), so BN1 and the regroup DMA run once per
   4 channels.
 - BN1 scale a1 is folded into the Toeplitz weights, so BN1+ReLU is
   relu(psum + c1): one ScalarE activation per (quad, phase).  Same
   fold for BN2 into the pointwise weights; BN2+ReLU is a single
   add+max tensor_scalar split across DVE and ScalarE.
 - A per-quad SBUF->SBUF DMA on the GpSimd SWDGE queue (distributes
   evenly over all 16 DMA engines, unlike small HWDGE transfers)
   regroups (pp,c2,ho,wo)-partitions into pure-channel partitions.
 - Two d-phases (out d 0..23 | 24..47) pipeline PW of phase A under
   DW of phase B; PW chunks are interleaved between DW quads.
 - Output stays [f, positions] on device; host transposes to NDHWC.
"""

import sys

for _p in ("/opt/trn_rl_repo", "/opt/pypackages"):
    if _p not in sys.path:
        sys.path.insert(0, _p)

import numpy as np
import ml_dtypes

import concourse.bass as bass
import concourse.tile as tile
from concourse import bacc, mybir
from concourse.bass_utils import run_bass_kernel_spmd

# ----- problem constants (hardcoded per spec) -----
B, D, H, W, C, F = 2, 48, 48, 48, 64, 128
EPS = 1e-3
N_CORES = 8
HQ = H // 4                       # 12 h-rows per core
NP = C // 2                       # 32 channel-pairs
NQ = NP // 2                      # 16 quads (2 pairs each)
POH, POW = 4, 8                   # patch out edges (ho, wo)
PIH, PIW = POH + 2, POW + 2       # 6, 10
TH = HQ // POH                    # 3 h-tiles
TW = W // POW                     # 6 w-tiles
MP = 64                           # output partitions per pair
KP = 2 * PIH * PIW                # 120 input partitions
NB_ = POH * POW                   # 32 (ho,wo) blocks
DI = D + 2                        # 50 padded d slices
NTT = TH * TW                     # 18 (th,tw) tiles
DA = 24                           # d-phase split (outputs per phase)
NA = DA * NTT                     # 432 moving cols per (pair,dz,phase)
GRP = 8                           # pairs per x tile
NG = NP // GRP                    # 4 groups
ZC = 512                          # pointwise chunk (PSUM cols, 1 bank)
NCH = NB_ * NA // ZC              # 27 pw chunks per phase
NPOS = 2 * NB_ * NA               # 27648 positions per core

BF16 = mybir.dt.bfloat16
F32 = mybir.dt.float32
RELU = mybir.ActivationFunctionType.Relu
ADD = mybir.AluOpType.add
MAX = mybir.AluOpType.max

_COMPILED = None


def _build_bass():
    nc = bacc.Bacc("TRN2", target_bir_lowering=False, debug=False,
                   num_devices=N_CORES)

    xt_d = nc.dram_tensor("xt", [NP, KP, DI, NTT], BF16,
                          kind="ExternalInput").ap()
    wt_d = nc.dram_tensor("wt", [KP, NP, 3, MP], BF16,
                          kind="ExternalInput").ap()
    pw_d = nc.dram_tensor("pwk", [C, F], BF16, kind="ExternalInput").ap()
    c1_d = nc.dram_tensor("c1b", [2 * MP, NQ], F32,
                          kind="ExternalInput").ap()
    c2_d = nc.dram_tensor("c2b", [F, 1], F32, kind="ExternalInput").ap()
    z_d = nc.dram_tensor("z", [F, NPOS], BF16, kind="ExternalOutput").ap()

    with tile.TileContext(nc) as tc:
        with (
            tc.tile_pool(name="consts", bufs=1) as consts,
            tc.tile_pool(name="xt", bufs=NG) as xt_pool,
            tc.tile_pool(name="Y", bufs=1) as Y_pool,
            tc.tile_pool(name="yg", bufs=4) as yg_pool,
            tc.tile_pool(name="zbuf", bufs=4) as z_pool,
        ):
            pw_sb = consts.tile([C, F], BF16)
            c1_sb = consts.tile([2 * MP, NQ], F32)
            c2_sb = consts.tile([F, 1], F32)
            wt_sb = consts.tile([KP, NP, 3, MP], BF16)

            xg = [xt_pool.tile([KP, GRP, DI, NTT], BF16, tag="xg",
                               name=f"xg_{g}")
                  for g in range(NG)]

            # input DMAs: x on the SP ring (phase-A d-range first, in
            # group order so compute can start after the first pairs),
            # weights/consts on the ACT ring
            xsrc = xt_d.rearrange("(g p) k d t -> k g p d t", g=NG)
            nc.scalar.dma_start(wt_sb[:, 0:2], wt_d[:, 0:2])
            nc.sync.dma_start(xg[0][:, 0:2, 0:DA + 2],
                              xsrc[:, 0, 0:2, 0:DA + 2])
            nc.scalar.dma_start(c1_sb[:], c1_d[:])
            nc.scalar.dma_start(c2_sb[:], c2_d[:])
            nc.scalar.dma_start(pw_sb[:], pw_d[:])
            nc.sync.dma_start(xg[0][:, 2:GRP, 0:DA + 2],
                              xsrc[:, 0, 2:GRP, 0:DA + 2])
            nc.scalar.dma_start(wt_sb[:, 2:GRP], wt_d[:, 2:GRP])
            nc.sync.dma_start(xg[1][:, :, 0:DA + 2], xsrc[:, 1, :, 0:DA + 2])
            nc.scalar.dma_start(wt_sb[:, GRP:NP], wt_d[:, GRP:NP])
            for g in (2, 3):
                nc.sync.dma_start(xg[g][:, :, 0:DA + 2],
                                  xsrc[:, g, :, 0:DA + 2])
            for g in range(NG):
                nc.sync.dma_start(xg[g][:, :, DA + 2:DI],
                                  xsrc[:, g, :, DA + 2:DI])

            # depthwise output, channel-partition layout, per phase
            Yt = [Y_pool.tile([C, NB_, NA], BF16, tag=f"Y{i}", name=f"Y{i}")
                  for i in range(2)]

            with (
                tc.tile_pool(name="psdw", bufs=3, space="PSUM") as ps_pool,
                tc.tile_pool(name="pspw", bufs=4, space="PSUM") as pw_pool,
            ):
                pw_fifo = []
                zq = {"n": 0}

                def flush_z():
                    k = zq["n"]
                    if not k:
                        return
                    nc.sync.dma_start(
                        z_d[:, zq["off"]:zq["off"] + k * ZC],
                        zq["t"][:, 0:k].rearrange("f s r -> f (s r)"))
                    zq["n"] = 0

                def emit_pw(ph, q, tail=False):
                    off = ph * NB_ * NA + q * ZC
                    Yv = Yt[ph][:].rearrange("c a b -> c (a b)")
                    pps = pw_pool.tile([F, ZC], F32, tag="pwps",
                                       name=f"pps_{ph}_{q}")
                    nc.tensor.matmul(pps[:], pw_sb[:],
                                     Yv[:, q * ZC:(q + 1) * ZC],
                                     start=True, stop=True)
                    if zq["n"] == 0:
                        zq["t"] = z_pool.tile([F, 4, ZC], BF16, tag="zt",
                                              name=f"zt_{ph}_{q}")
                        zq["off"] = off
                    s = zq["n"]
                    zt = zq["t"]
                    on_act = (q % 2 == 0) if tail else (q % 3 == 2)
                    if on_act:
                        nc.scalar.activation(zt[:, s], pps[:], RELU,
                                             bias=c2_sb[:, 0:1])
                    else:
                        nc.vector.tensor_scalar(zt[:, s], pps[:],
                                                c2_sb[:, 0:1], 0.0,
                                                ADD, MAX)
                    zq["n"] = s + 1
                    if zq["n"] == 4:
                        flush_z()

                for ph in range(2):
                    d0 = ph * DA
                    for j in range(NQ):
                        g, jg = j // 4, j % 4
                        psq = ps_pool.tile([2 * MP, ZC], F32, tag="ps",
                                           name=f"ps_{ph}_{j}")
                        for s in range(2):
                            p = 2 * j + s
                            for dz in range(3):
                                rhs = xg[g][:, 2 * jg + s,
                                            d0 + dz:d0 + dz + DA]
                                nc.tensor.matmul(
                                    psq[s * MP:(s + 1) * MP, 0:NA],
                                    wt_sb[:, p, dz], rhs,
                                    start=(dz == 0), stop=(dz == 2))
                        ygq = yg_pool.tile([2 * MP, NA], BF16, tag="yg",
                                           name=f"yg_{ph}_{j}")
                        nc.scalar.activation(
                            ygq[:], psq[:, 0:NA], RELU,
                            bias=c1_sb[:, j:j + 1])
                        # regroup (pp,c2,ho,wo)->channel partitions on
                        # the Pool SWDGE queue (even engine spread)
                        nc.gpsimd.dma_start(Yt[ph][4 * j: 4 * j + 4],
                                            ygq[:])
                        # interleave prev-phase PW under this DW
                        for _ in range(2):
                            if pw_fifo:
                                emit_pw(*pw_fifo.pop(0))
                    for q in range(NCH):
                        pw_fifo.append((ph, q))
                    if ph == 1:
                        while pw_fifo:
                            emit_pw(*pw_fifo.pop(0), tail=True)
                        flush_z()

    nc.compile()
    return nc


def _prep_inputs(x, dw_kernel, dw_bias, bn1_gamma, bn1_beta, bn1_mean,
                 bn1_var, pw_kernel, pw_bias, bn2_gamma, bn2_beta, bn2_mean,
                 bn2_var):
    """Build per-core input maps (numpy only, off the device clock)."""
    x = np.asarray(x, np.float32)
    dw = np.asarray(dw_kernel, np.float32)[:, :, :, 0, :]     # [3,3,3,C]
    a1 = np.asarray(bn1_gamma, np.float32) / np.sqrt(
        np.asarray(bn1_var, np.float32) + EPS)
    c1 = a1 * (np.asarray(dw_bias, np.float32)
               - np.asarray(bn1_mean, np.float32)) \
        + np.asarray(bn1_beta, np.float32)
    a2 = np.asarray(bn2_gamma, np.float32) / np.sqrt(
        np.asarray(bn2_var, np.float32) + EPS)
    c2 = a2 * (np.asarray(pw_bias, np.float32)
               - np.asarray(bn2_mean, np.float32)) \
        + np.asarray(bn2_beta, np.float32)

    # Toeplitz weights [KP, NP, 3, MP], a1 prefolded, k-major
    aw = dw * a1[None, None, None, :]                         # [3,3,3,C]
    wt = np.zeros((2, PIH, PIW, NP, 3, 2, POH, POW), np.float32)
    c2i = np.arange(2)[:, None, None]
    hoi = np.arange(POH)[None, :, None]
    woi = np.arange(POW)[None, None, :]
    for dy in range(3):
        for dx in range(3):
            # value for [c2, ho, wo, p, dz] = aw[dz, dy, dx, 2p+c2]
            val = aw[:, dy, dx, :].reshape(3, NP, 2)          # [dz, p, c2]
            val = val.transpose(2, 1, 0)[:, None, None]       # [2,1,1,NP,3]
            wt[c2i, dy + hoi, dx + woi, :, :, c2i, hoi, woi] = val
    wt = wt.reshape(KP, NP, 3, MP).astype(ml_dtypes.bfloat16)

    # c1b[(pp,c2,ho,wo), j] = c1[4j + 2pp + c2]
    c1b = np.ascontiguousarray(
        np.repeat(c1.reshape(NQ, 4).T, NB_, axis=0).reshape(2 * MP, NQ)
    ).astype(np.float32)
    pwk = (np.asarray(pw_kernel, np.float32)
           * a2[None, :]).astype(ml_dtypes.bfloat16)
    c2b = c2.reshape(F, 1).astype(np.float32)

    # x padded once globally: [B, D+2, H+2, W+2, C]
    xp = np.zeros((B, D + 2, H + 2, W + 2, C), np.float32)
    xp[:, 1:-1, 1:-1, 1:-1, :] = x

    widx = (np.arange(TW)[:, None] * POW + np.arange(PIW)[None, :])
    hidx = (np.arange(TH)[:, None] * POH + np.arange(PIH)[None, :])
    in_maps = []
    for core in range(N_CORES):
        b, hq = core // 4, core % 4
        slab = xp[b, :, hq * HQ: hq * HQ + HQ + 2]        # [50, 14, 50, C]
        t = slab[:, :, widx.ravel()].reshape(DI, HQ + 2, TW, PIW, C)
        t = t[:, hidx.ravel()].reshape(DI, TH, PIH, TW, PIW, C)
        # [d, th, hy, tw, wx, c] -> [c, hy, wx, d, th, tw]
        t = t.transpose(5, 2, 4, 0, 1, 3)
        xt = t.reshape(NP, 2, PIH, PIW, DI, NTT).reshape(NP, KP, DI, NTT)
        in_maps.append({
            "xt": np.ascontiguousarray(xt).astype(ml_dtypes.bfloat16),
            "wt": wt, "pwk": pwk, "c1b": c1b, "c2b": c2b,
        })
    return in_maps


def _gather_output(results):
    z = np.empty((B, D, H, W, F), np.float32)
    for core in range(N_CORES):
        b, hq = core // 4, core % 4
        zc = np.asarray(results[core]["z"], dtype=np.float32)  # [F, NPOS]
        for ph in range(2):
            zp = zc[:, ph * NB_ * NA:(ph + 1) * NB_ * NA]
            v = zp.reshape(F, POH, POW, DA, TH, TW)
            # [f, ho, wo, d, th, tw] -> [d, th, ho, tw, wo, f]
            v = v.transpose(3, 4, 1, 5, 2, 0)
            z[b, ph * DA:(ph + 1) * DA, hq * HQ: hq * HQ + HQ] = \
                v.reshape(DA, HQ, W, F)
    return z


def kernel(**inputs):
    global _COMPILED
    if _COMPILED is None:
        _COMPILED = _build_bass()
    in_maps = _prep_inputs(**inputs)
    res = run_bass_kernel_spmd(_COMPILED, in_maps,
                               core_ids=list(range(N_CORES)))
    return _gather_output(res.results)


if __name__ == "__main__":
    pass


# revision 21
# speedup vs baseline: 1.1607x; 1.0231x over previous
"""Trainium2 Bass kernel for DepthwiseSeparableConv3d (inference).

Problem: x[2,48,48,48,64] -> dw3x3x3 depthwise + BN + ReLU -> 1x1x1 conv
(64->128) + BN + ReLU -> z[2,48,48,48,128], all f32.

Strategy (8 NeuronCores, data-parallel over (b, h-quarter) slabs):
 - Each core owns batch b = core//4 and h-rows [12*hq, 12*hq+12) for
   hq = core%4, full D and W, with SAME-pad halos baked in on host.
 - Depthwise conv runs on TensorE as a 2D block-Toeplitz matmul:
   K = 120 partitions = (2 channels x 6x10 (h,w) input patch),
   M = 64 partitions  = (2 channels x 4x8 (h,w) output patch).
   The 9 (dy,dx) taps live in the Toeplitz weight; the 3 dz taps are
   PSUM-accumulated matmuls against d-shifted views of the same SBUF
   tile.  TWO channel-pairs share each PSUM tile via PE column tiling
   (tile_position cols 0/64), so BN1 and the regroup DMA run once per
   4 channels.
 - BN1 scale a1 is folded into the Toeplitz weights, so BN1+ReLU is
   relu(psum + c1): one ScalarE activation per (quad, phase).  Same
   fold for BN2 into the pointwise weights; BN2+ReLU is a single
   add+max tensor_scalar split across DVE and ScalarE.
 - A per-quad SBUF->SBUF DMA on the GpSimd SWDGE queue (distributes
   evenly over all 16 DMA engines, unlike small HWDGE transfers)
   regroups (pp,c2,ho,wo)-partitions into pure-channel partitions.
 - Two d-phases (out d 0..23 | 24..47) pipeline PW of phase A under
   DW of phase B; PW chunks are interleaved between DW quads.
 - Output stays [f, positions] on device; host transposes to NDHWC.
"""

import sys

for _p in ("/opt/trn_rl_repo", "/opt/pypackages"):
    if _p not in sys.path:
        sys.path.insert(0, _p)

import numpy as np
import ml_dtypes

import concourse.bass as bass
import concourse.tile as tile
from concourse import bacc, mybir
from concourse.bass_utils import run_bass_kernel_spmd

# ----- problem constants (hardcoded per spec) -----
B, D, H, W, C, F = 2, 48, 48, 48, 64, 128
EPS = 1e-3
N_CORES = 8
HQ = H // 4                       # 12 h-rows per core
NP = C // 2                       # 32 channel-pairs
NQ = NP // 2                      # 16 quads (2 pairs each)
POH, POW = 4, 8                   # patch out edges (ho, wo)
PIH, PIW = POH + 2, POW + 2       # 6, 10
TH = HQ // POH                    # 3 h-tiles
TW = W // POW                     # 6 w-tiles
MP = 64                           # output partitions per pair
KP = 2 * PIH * PIW                # 120 input partitions
NB_ = POH * POW                   # 32 (ho,wo) blocks
DI = D + 2                        # 50 padded d slices
NTT = TH * TW                     # 18 (th,tw) tiles
DA = 24                           # d-phase split (outputs per phase)
NA = DA * NTT                     # 432 moving cols per (pair,dz,phase)
GRP = 8                           # pairs per x tile
NG = NP // GRP                    # 4 groups
ZC = 512                          # pointwise chunk (PSUM cols, 1 bank)
NCH = NB_ * NA // ZC              # 27 pw chunks per phase
NPOS = 2 * NB_ * NA               # 27648 positions per core

BF16 = mybir.dt.bfloat16
F32 = mybir.dt.float32
RELU = mybir.ActivationFunctionType.Relu
ADD = mybir.AluOpType.add
MAX = mybir.AluOpType.max

_COMPILED = None


def _build_bass():
    nc = bacc.Bacc("TRN2", target_bir_lowering=False, debug=False,
                   num_devices=N_CORES)

    xt_d = nc.dram_tensor("xt", [NG, KP, GRP, DI, NTT], BF16,
                          kind="ExternalInput").ap()
    wt_d = nc.dram_tensor("wt", [KP, NP, 3, MP], BF16,
                          kind="ExternalInput").ap()
    pw_d = nc.dram_tensor("pwk", [C, F], BF16, kind="ExternalInput").ap()
    c1_d = nc.dram_tensor("c1b", [2 * MP, NQ], F32,
                          kind="ExternalInput").ap()
    c2_d = nc.dram_tensor("c2b", [F, 1], F32, kind="ExternalInput").ap()
    z_d = nc.dram_tensor("z", [F, NPOS], BF16, kind="ExternalOutput").ap()

    with tile.TileContext(nc) as tc:
        with (
            tc.tile_pool(name="consts", bufs=1) as consts,
            tc.tile_pool(name="xt", bufs=NG) as xt_pool,
            tc.tile_pool(name="Y", bufs=1) as Y_pool,
            tc.tile_pool(name="yg", bufs=4) as yg_pool,
            tc.tile_pool(name="zbuf", bufs=4) as z_pool,
        ):
            pw_sb = consts.tile([C, F], BF16)
            c1_sb = consts.tile([2 * MP, NQ], F32)
            c2_sb = consts.tile([F, 1], F32)
            wt_sb = consts.tile([KP, NP, 3, MP], BF16)

            xg = [xt_pool.tile([KP, GRP, DI, NTT], BF16, tag="xg",
                               name=f"xg_{g}")
                  for g in range(NG)]

            # input DMAs: x on the SP ring, k-major so each descriptor
            # is a 7.2KB per-partition run (4 pairs per DMA, in pair
            # order so compute starts early); weights/consts on ACT
            nc.scalar.dma_start(wt_sb[:, 0:4], wt_d[:, 0:4])
            nc.sync.dma_start(xg[0][:, 0:4], xt_d[0, :, 0:4])
            nc.scalar.dma_start(c1_sb[:], c1_d[:])
            nc.scalar.dma_start(c2_sb[:], c2_d[:])
            nc.scalar.dma_start(pw_sb[:], pw_d[:])
            nc.sync.dma_start(xg[0][:, 4:GRP], xt_d[0, :, 4:GRP])
            nc.scalar.dma_start(wt_sb[:, 4:GRP], wt_d[:, 4:GRP])
            nc.sync.dma_start(xg[1][:, 0:4], xt_d[1, :, 0:4])
            nc.scalar.dma_start(wt_sb[:, GRP:NP], wt_d[:, GRP:NP])
            nc.sync.dma_start(xg[1][:, 4:GRP], xt_d[1, :, 4:GRP])
            for g in (2, 3):
                nc.sync.dma_start(xg[g][:, 0:4], xt_d[g, :, 0:4])
                nc.sync.dma_start(xg[g][:, 4:GRP], xt_d[g, :, 4:GRP])

            # depthwise output, channel-partition layout, per phase
            Yt = [Y_pool.tile([C, NB_, NA], BF16, tag=f"Y{i}", name=f"Y{i}")
                  for i in range(2)]

            with (
                tc.tile_pool(name="psdw", bufs=3, space="PSUM") as ps_pool,
                tc.tile_pool(name="pspw", bufs=4, space="PSUM") as pw_pool,
            ):
                pw_fifo = []
                zq = {"n": 0}

                def flush_z():
                    k = zq["n"]
                    if not k:
                        return
                    nc.sync.dma_start(
                        z_d[:, zq["off"]:zq["off"] + k * ZC],
                        zq["t"][:, 0:k].rearrange("f s r -> f (s r)"))
                    zq["n"] = 0

                def emit_pw(ph, q, tail=False):
                    off = ph * NB_ * NA + q * ZC
                    Yv = Yt[ph][:].rearrange("c a b -> c (a b)")
                    pps = pw_pool.tile([F, ZC], F32, tag="pwps",
                                       name=f"pps_{ph}_{q}")
                    nc.tensor.matmul(pps[:], pw_sb[:],
                                     Yv[:, q * ZC:(q + 1) * ZC],
                                     start=True, stop=True)
                    if zq["n"] == 0:
                        zq["t"] = z_pool.tile([F, 4, ZC], BF16, tag="zt",
                                              name=f"zt_{ph}_{q}")
                        zq["off"] = off
                    s = zq["n"]
                    zt = zq["t"]
                    on_act = (q % 2 == 0) if tail else (q % 3 == 2)
                    if on_act:
                        nc.scalar.activation(zt[:, s], pps[:], RELU,
                                             bias=c2_sb[:, 0:1])
                    else:
                        nc.vector.tensor_scalar(zt[:, s], pps[:],
                                                c2_sb[:, 0:1], 0.0,
                                                ADD, MAX)
                    zq["n"] = s + 1
                    if zq["n"] == 4:
                        flush_z()

                for ph in range(2):
                    d0 = ph * DA
                    for j in range(NQ):
                        g, jg = j // 4, j % 4
                        psq = ps_pool.tile([2 * MP, ZC], F32, tag="ps",
                                           name=f"ps_{ph}_{j}")
                        for s in range(2):
                            p = 2 * j + s
                            for dz in range(3):
                                rhs = xg[g][:, 2 * jg + s,
                                            d0 + dz:d0 + dz + DA]
                                nc.tensor.matmul(
                                    psq[s * MP:(s + 1) * MP, 0:NA],
                                    wt_sb[:, p, dz], rhs,
                                    start=(dz == 0), stop=(dz == 2))
                        ygq = yg_pool.tile([2 * MP, NA], BF16, tag="yg",
                                           name=f"yg_{ph}_{j}")
                        nc.scalar.activation(
                            ygq[:], psq[:, 0:NA], RELU,
                            bias=c1_sb[:, j:j + 1])
                        # regroup (pp,c2,ho,wo)->channel partitions on
                        # the Pool SWDGE queue (even engine spread)
                        nc.gpsimd.dma_start(Yt[ph][4 * j: 4 * j + 4],
                                            ygq[:])
                        # interleave prev-phase PW under this DW
                        for _ in range(2):
                            if pw_fifo:
                                emit_pw(*pw_fifo.pop(0))
                    for q in range(NCH):
                        pw_fifo.append((ph, q))
                    if ph == 1:
                        while pw_fifo:
                            emit_pw(*pw_fifo.pop(0), tail=True)
                        flush_z()

    nc.compile()
    return nc


def _prep_inputs(x, dw_kernel, dw_bias, bn1_gamma, bn1_beta, bn1_mean,
                 bn1_var, pw_kernel, pw_bias, bn2_gamma, bn2_beta, bn2_mean,
                 bn2_var):
    """Build per-core input maps (numpy only, off the device clock)."""
    x = np.asarray(x, np.float32)
    dw = np.asarray(dw_kernel, np.float32)[:, :, :, 0, :]     # [3,3,3,C]
    a1 = np.asarray(bn1_gamma, np.float32) / np.sqrt(
        np.asarray(bn1_var, np.float32) + EPS)
    c1 = a1 * (np.asarray(dw_bias, np.float32)
               - np.asarray(bn1_mean, np.float32)) \
        + np.asarray(bn1_beta, np.float32)
    a2 = np.asarray(bn2_gamma, np.float32) / np.sqrt(
        np.asarray(bn2_var, np.float32) + EPS)
    c2 = a2 * (np.asarray(pw_bias, np.float32)
               - np.asarray(bn2_mean, np.float32)) \
        + np.asarray(bn2_beta, np.float32)

    # Toeplitz weights [KP, NP, 3, MP], a1 prefolded, k-major
    aw = dw * a1[None, None, None, :]                         # [3,3,3,C]
    wt = np.zeros((2, PIH, PIW, NP, 3, 2, POH, POW), np.float32)
    c2i = np.arange(2)[:, None, None]
    hoi = np.arange(POH)[None, :, None]
    woi = np.arange(POW)[None, None, :]
    for dy in range(3):
        for dx in range(3):
            # value for [c2, ho, wo, p, dz] = aw[dz, dy, dx, 2p+c2]
            val = aw[:, dy, dx, :].reshape(3, NP, 2)          # [dz, p, c2]
            val = val.transpose(2, 1, 0)[:, None, None]       # [2,1,1,NP,3]
            wt[c2i, dy + hoi, dx + woi, :, :, c2i, hoi, woi] = val
    wt = wt.reshape(KP, NP, 3, MP).astype(ml_dtypes.bfloat16)

    # c1b[(pp,c2,ho,wo), j] = c1[4j + 2pp + c2]
    c1b = np.ascontiguousarray(
        np.repeat(c1.reshape(NQ, 4).T, NB_, axis=0).reshape(2 * MP, NQ)
    ).astype(np.float32)
    pwk = (np.asarray(pw_kernel, np.float32)
           * a2[None, :]).astype(ml_dtypes.bfloat16)
    c2b = c2.reshape(F, 1).astype(np.float32)

    # x padded once globally: [B, D+2, H+2, W+2, C]
    xp = np.zeros((B, D + 2, H + 2, W + 2, C), np.float32)
    xp[:, 1:-1, 1:-1, 1:-1, :] = x

    widx = (np.arange(TW)[:, None] * POW + np.arange(PIW)[None, :])
    hidx = (np.arange(TH)[:, None] * POH + np.arange(PIH)[None, :])
    in_maps = []
    for core in range(N_CORES):
        b, hq = core // 4, core % 4
        slab = xp[b, :, hq * HQ: hq * HQ + HQ + 2]        # [50, 14, 50, C]
        t = slab[:, :, widx.ravel()].reshape(DI, HQ + 2, TW, PIW, C)
        t = t[:, hidx.ravel()].reshape(DI, TH, PIH, TW, PIW, C)
        # [d, th, hy, tw, wx, c] -> [c, hy, wx, d, th, tw]
        t = t.transpose(5, 2, 4, 0, 1, 3)
        xt = t.reshape(NP, KP, DI * NTT)
        # k-major groups: [NG, KP, GRP, DI*NTT]
        xt = xt.reshape(NG, GRP, KP, DI * NTT).transpose(0, 2, 1, 3)
        xt = xt.reshape(NG, KP, GRP, DI, NTT)
        in_maps.append({
            "xt": np.ascontiguousarray(xt).astype(ml_dtypes.bfloat16),
            "wt": wt, "pwk": pwk, "c1b": c1b, "c2b": c2b,
        })
    return in_maps


def _gather_output(results):
    z = np.empty((B, D, H, W, F), np.float32)
    for core in range(N_CORES):
        b, hq = core // 4, core % 4
        zc = np.asarray(results[core]["z"], dtype=np.float32)  # [F, NPOS]
        for ph in range(2):
            zp = zc[:, ph * NB_ * NA:(ph + 1) * NB_ * NA]
            v = zp.reshape(F, POH, POW, DA, TH, TW)
            # [f, ho, wo, d, th, tw] -> [d, th, ho, tw, wo, f]
            v = v.transpose(3, 4, 1, 5, 2, 0)
            z[b, ph * DA:(ph + 1) * DA, hq * HQ: hq * HQ + HQ] = \
                v.reshape(DA, HQ, W, F)
    return z


def kernel(**inputs):
    global _COMPILED
    if _COMPILED is None:
        _COMPILED = _build_bass()
    in_maps = _prep_inputs(**inputs)
    res = run_bass_kernel_spmd(_COMPILED, in_maps,
                               core_ids=list(range(N_CORES)))
    return _gather_output(res.results)


if __name__ == "__main__":
    pass


# revision 24
# speedup vs baseline: 1.2136x; 1.0455x over previous
"""Trainium2 Bass kernel for DepthwiseSeparableConv3d (inference).

Problem: x[2,48,48,48,64] -> dw3x3x3 depthwise + BN + ReLU -> 1x1x1 conv
(64->128) + BN + ReLU -> z[2,48,48,48,128], all f32.

Strategy (8 NeuronCores, data-parallel over (b, h-quarter) slabs):
 - Each core owns batch b = core//4 and h-rows [12*hq, 12*hq+12) for
   hq = core%4, full D and W, with SAME-pad halos baked in on host.
 - Depthwise conv runs on TensorE as a 2D block-Toeplitz matmul:
   K = 120 partitions = (2 channels x 6x10 (h,w) input patch),
   M = 64 partitions  = (2 channels x 4x8 (h,w) output patch).
   The 9 (dy,dx) taps live in the Toeplitz weight; the 3 dz taps are
   PSUM-accumulated matmuls against d-shifted views of the same SBUF
   tile.  TWO channel-pairs share each PSUM tile via PE column tiling
   (tile_position cols 0/64), so BN1 and the regroup DMA run once per
   4 channels.
 - BN1 scale a1 is folded into the Toeplitz weights, so BN1+ReLU is
   relu(psum + c1): one ScalarE activation per (quad, phase).  Same
   fold for BN2 into the pointwise weights; BN2+ReLU is a single
   add+max tensor_scalar split across DVE and ScalarE.
 - A per-quad SBUF->SBUF DMA on the GpSimd SWDGE queue (distributes
   evenly over all 16 DMA engines, unlike small HWDGE transfers)
   regroups (pp,c2,ho,wo)-partitions into pure-channel partitions.
 - Two d-phases (out d 0..23 | 24..47) pipeline PW of phase A under
   DW of phase B; PW chunks are interleaved between DW quads.
 - Output stays [f, positions] on device; host transposes to NDHWC.
"""

import sys

for _p in ("/opt/trn_rl_repo", "/opt/pypackages"):
    if _p not in sys.path:
        sys.path.insert(0, _p)

import numpy as np
import ml_dtypes

import concourse.bass as bass
import concourse.tile as tile
from concourse import bacc, mybir
from concourse.bass_utils import run_bass_kernel_spmd

# ----- problem constants (hardcoded per spec) -----
B, D, H, W, C, F = 2, 48, 48, 48, 64, 128
EPS = 1e-3
N_CORES = 8
HQ = H // 4                       # 12 h-rows per core
NP = C // 2                       # 32 channel-pairs
NQ = NP // 2                      # 16 quads (2 pairs each)
POH, POW = 4, 8                   # patch out edges (ho, wo)
PIH, PIW = POH + 2, POW + 2       # 6, 10
TH = HQ // POH                    # 3 h-tiles
TW = W // POW                     # 6 w-tiles
MP = 64                           # output partitions per pair
KP = 2 * PIH * PIW                # 120 input partitions
NB_ = POH * POW                   # 32 (ho,wo) blocks
DI = D + 2                        # 50 padded d slices
NTT = TH * TW                     # 18 (th,tw) tiles
DA = 24                           # d-phase split (outputs per phase)
NA = DA * NTT                     # 432 moving cols per (pair,dz,phase)
GRP = 8                           # pairs per x tile
NG = NP // GRP                    # 4 groups
ZC = 512                          # pointwise chunk (PSUM cols, 1 bank)
NCH = NB_ * NA // ZC              # 27 pw chunks per phase
NPOS = 2 * NB_ * NA               # 27648 positions per core

BF16 = mybir.dt.bfloat16
F32 = mybir.dt.float32
RELU = mybir.ActivationFunctionType.Relu
ADD = mybir.AluOpType.add
MAX = mybir.AluOpType.max

_COMPILED = None


def _build_bass():
    nc = bacc.Bacc("TRN2", target_bir_lowering=False, debug=False,
                   num_devices=N_CORES)

    xt_d = nc.dram_tensor("xt", [NG, KP, GRP, DI, NTT], BF16,
                          kind="ExternalInput").ap()
    wt_d = nc.dram_tensor("wt", [KP, NP, 3, MP], BF16,
                          kind="ExternalInput").ap()
    pw_d = nc.dram_tensor("pwk", [C, F], BF16, kind="ExternalInput").ap()
    c1_d = nc.dram_tensor("c1b", [2 * MP, NQ], F32,
                          kind="ExternalInput").ap()
    c2_d = nc.dram_tensor("c2b", [F, 1], F32, kind="ExternalInput").ap()
    z_d = nc.dram_tensor("z", [F, NPOS], BF16, kind="ExternalOutput").ap()

    with tile.TileContext(nc) as tc:
        with (
            tc.tile_pool(name="consts", bufs=1) as consts,
            tc.tile_pool(name="xt", bufs=NG) as xt_pool,
            tc.tile_pool(name="Y", bufs=1) as Y_pool,
            tc.tile_pool(name="yg", bufs=6) as yg_pool,
            tc.tile_pool(name="zbuf", bufs=4) as z_pool,
        ):
            pw_sb = consts.tile([C, F], BF16)
            c1_sb = consts.tile([2 * MP, NQ], F32)
            c2_sb = consts.tile([F, 1], F32)
            wt_sb = consts.tile([KP, NP, 3, MP], BF16)

            xg = [xt_pool.tile([KP, GRP, DI, NTT], BF16, tag="xg",
                               name=f"xg_{g}")
                  for g in range(NG)]

            # input DMAs: x on the SP ring, k-major so each descriptor
            # is a 7.2KB per-partition run (4 pairs per DMA, in pair
            # order so compute starts early); weights/consts on ACT
            nc.scalar.dma_start(wt_sb[:, 0:4], wt_d[:, 0:4])
            nc.sync.dma_start(xg[0][:, 0:4], xt_d[0, :, 0:4])
            nc.scalar.dma_start(c1_sb[:], c1_d[:])
            nc.scalar.dma_start(c2_sb[:], c2_d[:])
            nc.scalar.dma_start(pw_sb[:], pw_d[:])
            nc.sync.dma_start(xg[0][:, 4:GRP], xt_d[0, :, 4:GRP])
            nc.scalar.dma_start(wt_sb[:, 4:GRP], wt_d[:, 4:GRP])
            nc.sync.dma_start(xg[1][:, 0:4], xt_d[1, :, 0:4])
            nc.scalar.dma_start(wt_sb[:, GRP:NP], wt_d[:, GRP:NP])
            nc.sync.dma_start(xg[1][:, 4:GRP], xt_d[1, :, 4:GRP])
            for g in (2, 3):
                nc.sync.dma_start(xg[g][:, 0:4], xt_d[g, :, 0:4])
                nc.sync.dma_start(xg[g][:, 4:GRP], xt_d[g, :, 4:GRP])

            # depthwise output, channel-partition layout, per phase
            Yt = [Y_pool.tile([C, NB_, NA], BF16, tag=f"Y{i}", name=f"Y{i}")
                  for i in range(2)]

            with (
                tc.tile_pool(name="psdw", bufs=4, space="PSUM") as ps_pool,
                tc.tile_pool(name="pspw", bufs=4, space="PSUM") as pw_pool,
            ):
                pw_fifo = []
                zq = {"n": 0}

                def flush_z():
                    k = zq["n"]
                    if not k:
                        return
                    nc.sync.dma_start(
                        z_d[:, zq["off"]:zq["off"] + k * ZC],
                        zq["t"][:, 0:k].rearrange("f s r -> f (s r)"))
                    zq["n"] = 0

                def emit_pw(ph, q, tail=False):
                    off = ph * NB_ * NA + q * ZC
                    Yv = Yt[ph][:].rearrange("c a b -> c (a b)")
                    pps = pw_pool.tile([F, ZC], F32, tag="pwps",
                                       name=f"pps_{ph}_{q}")
                    nc.tensor.matmul(pps[:], pw_sb[:],
                                     Yv[:, q * ZC:(q + 1) * ZC],
                                     start=True, stop=True)
                    if zq["n"] == 0:
                        zq["t"] = z_pool.tile([F, 4, ZC], BF16, tag="zt",
                                              name=f"zt_{ph}_{q}")
                        zq["off"] = off
                    s = zq["n"]
                    zt = zq["t"]
                    on_act = (q % 2 == 0) if tail else (q % 3 == 2)
                    if on_act:
                        nc.scalar.activation(zt[:, s], pps[:], RELU,
                                             bias=c2_sb[:, 0:1])
                    else:
                        nc.vector.tensor_scalar(zt[:, s], pps[:],
                                                c2_sb[:, 0:1], 0.0,
                                                ADD, MAX)
                    zq["n"] = s + 1
                    if zq["n"] == 4:
                        flush_z()

                for ph in range(2):
                    d0 = ph * DA
                    for j in range(NQ):
                        g, jg = j // 4, j % 4
                        psq = ps_pool.tile([2 * MP, ZC], F32, tag="ps",
                                           name=f"ps_{ph}_{j}")
                        for s in range(2):
                            p = 2 * j + s
                            for dz in range(3):
                                rhs = xg[g][:, 2 * jg + s,
                                            d0 + dz:d0 + dz + DA]
                                nc.tensor.matmul(
                                    psq[s * MP:(s + 1) * MP, 0:NA],
                                    wt_sb[:, p, dz], rhs,
                                    start=(dz == 0), stop=(dz == 2))
                        ygq = yg_pool.tile([2 * MP, NA], BF16, tag="yg",
                                           name=f"yg_{ph}_{j}")
                        nc.scalar.activation(
                            ygq[:], psq[:, 0:NA], RELU,
                            bias=c1_sb[:, j:j + 1])
                        # regroup (pp,c2,ho,wo)->channel partitions;
                        # alternate Pool SWDGE / ACT HWDGE queues so
                        # neither server falls behind DW production
                        eng = nc.gpsimd if j % 2 == 0 else nc.scalar
                        eng.dma_start(Yt[ph][4 * j: 4 * j + 4], ygq[:])
                        # interleave prev-phase PW under this DW
                        for _ in range(2):
                            if pw_fifo:
                                emit_pw(*pw_fifo.pop(0))
                    for q in range(NCH):
                        pw_fifo.append((ph, q))
                    if ph == 1:
                        while pw_fifo:
                            emit_pw(*pw_fifo.pop(0), tail=True)
                        flush_z()

    nc.compile()
    return nc


def _prep_inputs(x, dw_kernel, dw_bias, bn1_gamma, bn1_beta, bn1_mean,
                 bn1_var, pw_kernel, pw_bias, bn2_gamma, bn2_beta, bn2_mean,
                 bn2_var):
    """Build per-core input maps (numpy only, off the device clock)."""
    x = np.asarray(x, np.float32)
    dw = np.asarray(dw_kernel, np.float32)[:, :, :, 0, :]     # [3,3,3,C]
    a1 = np.asarray(bn1_gamma, np.float32) / np.sqrt(
        np.asarray(bn1_var, np.float32) + EPS)
    c1 = a1 * (np.asarray(dw_bias, np.float32)
               - np.asarray(bn1_mean, np.float32)) \
        + np.asarray(bn1_beta, np.float32)
    a2 = np.asarray(bn2_gamma, np.float32) / np.sqrt(
        np.asarray(bn2_var, np.float32) + EPS)
    c2 = a2 * (np.asarray(pw_bias, np.float32)
               - np.asarray(bn2_mean, np.float32)) \
        + np.asarray(bn2_beta, np.float32)

    # Toeplitz weights [KP, NP, 3, MP], a1 prefolded, k-major
    aw = dw * a1[None, None, None, :]                         # [3,3,3,C]
    wt = np.zeros((2, PIH, PIW, NP, 3, 2, POH, POW), np.float32)
    c2i = np.arange(2)[:, None, None]
    hoi = np.arange(POH)[None, :, None]
    woi = np.arange(POW)[None, None, :]
    for dy in range(3):
        for dx in range(3):
            # value for [c2, ho, wo, p, dz] = aw[dz, dy, dx, 2p+c2]
            val = aw[:, dy, dx, :].reshape(3, NP, 2)          # [dz, p, c2]
            val = val.transpose(2, 1, 0)[:, None, None]       # [2,1,1,NP,3]
            wt[c2i, dy + hoi, dx + woi, :, :, c2i, hoi, woi] = val
    wt = wt.reshape(KP, NP, 3, MP).astype(ml_dtypes.bfloat16)

    # c1b[(pp,c2,ho,wo), j] = c1[4j + 2pp + c2]
    c1b = np.ascontiguousarray(
        np.repeat(c1.reshape(NQ, 4).T, NB_, axis=0).reshape(2 * MP, NQ)
    ).astype(np.float32)
    pwk = (np.asarray(pw_kernel, np.float32)
           * a2[None, :]).astype(ml_dtypes.bfloat16)
    c2b = c2.reshape(F, 1).astype(np.float32)

    # x padded once globally: [B, D+2, H+2, W+2, C]
    xp = np.zeros((B, D + 2, H + 2, W + 2, C), np.float32)
    xp[:, 1:-1, 1:-1, 1:-1, :] = x

    widx = (np.arange(TW)[:, None] * POW + np.arange(PIW)[None, :])
    hidx = (np.arange(TH)[:, None] * POH + np.arange(PIH)[None, :])
    in_maps = []
    for core in range(N_CORES):
        b, hq = core // 4, core % 4
        slab = xp[b, :, hq * HQ: hq * HQ + HQ + 2]        # [50, 14, 50, C]
        t = slab[:, :, widx.ravel()].reshape(DI, HQ + 2, TW, PIW, C)
        t = t[:, hidx.ravel()].reshape(DI, TH, PIH, TW, PIW, C)
        # [d, th, hy, tw, wx, c] -> [c, hy, wx, d, th, tw]
        t = t.transpose(5, 2, 4, 0, 1, 3)
        xt = t.reshape(NP, KP, DI * NTT)
        # k-major groups: [NG, KP, GRP, DI*NTT]
        xt = xt.reshape(NG, GRP, KP, DI * NTT).transpose(0, 2, 1, 3)
        xt = xt.reshape(NG, KP, GRP, DI, NTT)
        in_maps.append({
            "xt": np.ascontiguousarray(xt).astype(ml_dtypes.bfloat16),
            "wt": wt, "pwk": pwk, "c1b": c1b, "c2b": c2b,
        })
    return in_maps


def _gather_output(results):
    z = np.empty((B, D, H, W, F), np.float32)
    for core in range(N_CORES):
        b, hq = core // 4, core % 4
        zc = np.asarray(results[core]["z"], dtype=np.float32)  # [F, NPOS]
        for ph in range(2):
            zp = zc[:, ph * NB_ * NA:(ph + 1) * NB_ * NA]
            v = zp.reshape(F, POH, POW, DA, TH, TW)
            # [f, ho, wo, d, th, tw] -> [d, th, ho, tw, wo, f]
            v = v.transpose(3, 4, 1, 5, 2, 0)
            z[b, ph * DA:(ph + 1) * DA, hq * HQ: hq * HQ + HQ] = \
                v.reshape(DA, HQ, W, F)
    return z


def kernel(**inputs):
    global _COMPILED
    if _COMPILED is None:
        _COMPILED = _build_bass()
    in_maps = _prep_inputs(**inputs)
    res = run_bass_kernel_spmd(_COMPILED, in_maps,
                               core_ids=list(range(N_CORES)))
    return _gather_output(res.results)


if __name__ == "__main__":
    pass
